# revision 4
# baseline (speedup 1.0000x reference)
"""AddAttention (Bahdanau) kernel for one TRN2 chip (8 NeuronCores).

Data-parallel over batch: B=32 -> 4 batch elements per core, weights
replicated. Per element (L=512, D=1024):
    q = query @ W_in^T ; c = context @ W_in2^T
    T = tanh(q + c + b_in + b_in2)
    scores = T @ W_out^T ; alpha = softmax(scores)
    mix = alpha @ context
    out = [mix, context] @ W_fc^T + b_fc
All matmuls run in bf16 (fp32 PSUM accumulation). Inputs stream in through
gpsimd casting DMAs (f32 DRAM -> bf16 SBUF). The PE wants the contraction
axis on partitions, so weights and query/context are transposed via the DMA
xbar in large batches (the xbar serializes on every transpose<->copy mode
switch, so batching matters); the per-l-chunk alpha transpose runs on the
TensorEngine (identity matmul) to keep it off the DMA critical path.
"""

import numpy as np

B, L, D = 32, 512, 1024
P = 128
NCORES = 8
BPC = B // NCORES  # batch elements per core

_cached = {}


# --- walrus workaround: this compiler build rejects >1 sync wait on several
# instruction classes. Rewrite the serialized BIR: surplus waits become
# single-wait NoOps immediately before the instruction on the same engine
# (engine streams are sequential, so semantics are unchanged).
def _split_waits_json(bir_bytes):
    import json

    bir = json.loads(bir_bytes)
    n = 0
    for f in bir["functions"]:
        for blk in f["blocks"]:
            out = []
            for inst in blk["instructions"]:
                si = inst.get("sync_info")
                waits = (si or {}).get("on_wait") or []
                if len(waits) > 1:
                    for w in waits[:-1]:
                        n += 1
                        out.append(
                            {
                                "debug": inst.get("debug"),
                                "engine": inst["engine"],
                                "ins": [],
                                "name": f"I-wsplit-{n}",
                                "opcode": "NoOp",
                                "outs": [],
                                "sync_info": {"on_update": [], "on_wait": [w]},
                                "text_hint": "wait_split",
                            }
                        )
                    si["on_wait"] = [waits[-1]]
                out.append(inst)
            blk["instructions"] = out
    return json.dumps(bir).encode()


def _build():
    import concourse.bass as bass
    import concourse.tile as tile
    from concourse import mybir
    from concourse.masks import make_identity
    from contextlib import ExitStack

    f32 = mybir.dt.float32
    b16 = mybir.dt.bfloat16
    Tanh = mybir.ActivationFunctionType.Tanh
    Exp = mybir.ActivationFunctionType.Exp
    ADD = mybir.AluOpType.add

    nc = bass.Bass("TRN2", target_bir_lowering=False, debug=False, num_devices=NCORES)

    q_ext = nc.dram_tensor("query", [BPC, L, D], f32, kind="ExternalInput").ap()
    c_ext = nc.dram_tensor("context", [BPC, L, D], f32, kind="ExternalInput").ap()
    w_in_ext = nc.dram_tensor("W_in", [D, D], f32, kind="ExternalInput").ap()
    b_in_ext = nc.dram_tensor("b_in", [D], f32, kind="ExternalInput").ap()
    w_in2_ext = nc.dram_tensor("W_in2", [D, D], f32, kind="ExternalInput").ap()
    b_in2_ext = nc.dram_tensor("b_in2", [D], f32, kind="ExternalInput").ap()
    w_out_ext = nc.dram_tensor("W_out", [L, D], f32, kind="ExternalInput").ap()
    w_fc_ext = nc.dram_tensor("W_fc", [D, 2 * D], f32, kind="ExternalInput").ap()
    b_fc_ext = nc.dram_tensor("b_fc", [D], f32, kind="ExternalInput").ap()
    out_ext = nc.dram_tensor("out", [BPC, L, D], f32, kind="ExternalOutput").ap()
    alpha_ext = nc.dram_tensor("alpha", [BPC, L, L], f32, kind="ExternalOutput").ap()

    with tile.TileContext(nc) as tc, ExitStack() as ctx:
        pool = lambda name, bufs, space="SBUF": ctx.enter_context(
            tc.tile_pool(name=name, bufs=bufs, space=space)
        )
        wpool = pool("weights", 1)
        wstg = pool("wstg", 1)      # bf16 natural weight staging (transpose src)
        qn_pool = pool("qn", 2)     # query natural bf16 (transpose src)
        cb_pool = pool("cb", 2)     # context natural bf16 (kept for stage D)
        qT_pool = pool("qT", 1)
        cT_pool = pool("cT", 2)     # read by deferred stage E
        tt_pool = pool("tt", 1)
        e_pool = pool("eb", 2)
        ab_pool = pool("ab", 2)
        aT_pool = pool("aT", 2)     # read by deferred stage D
        mT_pool = pool("mT", 1)
        zz_pool = pool("zz", 2)
        af_pool = pool("af", 2)
        of_pool = pool("of", 2)
        p_tt = pool("p_tt", 2, "PSUM")
        p_sc = pool("p_sc", 2, "PSUM")   # scores AND PE alpha-transpose tiles
        p_mx = pool("p_mx", 2, "PSUM")
        p_out = pool("p_out", 2, "PSUM")

        ident = wpool.tile([P, P], b16)
        make_identity(nc, ident[:])

        # final transposed weights: [P (col % P), cols//P, rows]
        w_inT = wpool.tile([P, 8, D], b16)    # W_in^T  [d, e]
        w_in2T = wpool.tile([P, 8, D], b16)   # W_in2^T [d, e]
        w_outT = wpool.tile([P, 8, L], b16)   # W_out^T [e, w]
        w_fcT = wpool.tile([P, 16, D], b16)   # W_fc^T  [f, dd]

        def prep_weight(dst, src, nrow_chunks, cols):
            # casting loads (gpsimd), batched; then xbar transposes, batched
            stage = wstg.tile([P, 8, 2 * D], b16, tag="wstg", name="wstg")
            for rc in range(nrow_chunks):
                nc.gpsimd.dma_start(
                    out=stage[:, rc, :cols], in_=src[rc * P : (rc + 1) * P, :]
                )
            for rc in range(nrow_chunks):
                nc.sync.dma_start_transpose(
                    dst[:, :, rc * P : (rc + 1) * P], stage[:, rc, :cols]
                )

        # ---- per-element stages ----
        def stageA(b):
            """Casting loads then batched xbar transposes -> QT/CT; CB kept."""
            QN = qn_pool.tile([P, 4, D], b16, tag="qn")
            CB = cb_pool.tile([P, 4, D], b16, tag="cb")
            QT = qT_pool.tile([P, 8, L], b16, tag="qT")
            CT = cT_pool.tile([P, 8, L], b16, tag="cT")
            for lc in range(4):
                ls = slice(lc * P, (lc + 1) * P)
                nc.gpsimd.dma_start(out=QN[:, lc, :], in_=q_ext[b, ls, :])
                nc.gpsimd.dma_start(out=CB[:, lc, :], in_=c_ext[b, ls, :])
            for lc in range(4):
                ls = slice(lc * P, (lc + 1) * P)
                nc.sync.dma_start_transpose(QT[:, :, ls], QN[:, lc, :])
                nc.sync.dma_start_transpose(CT[:, :, ls], CB[:, lc, :])
            return QT, CT, CB

        def stageB(b, QT, CT):
            """TT[e, l] = tanh(W_in q^T + W_in2 c^T + bias), e on partitions."""
            TT = tt_pool.tile([P, 8, L], b16, tag="tt")
            for ec in range(8):
                es = slice(ec * P, (ec + 1) * P)
                ps = p_tt.tile([P, L], f32, tag="p_tt")
                for dc in range(8):
                    nc.tensor.matmul(
                        ps[:], w_inT[:, dc, es], QT[:, dc, :],
                        start=(dc == 0), stop=False,
                    )
                for dc in range(8):
                    nc.tensor.matmul(
                        ps[:], w_in2T[:, dc, es], CT[:, dc, :],
                        start=False, stop=(dc == 7),
                    )
                nc.scalar.activation(
                    TT[:, ec, :], ps[:], Tanh, bias=bsum[:, ec : ec + 1]
                )
            return TT

        def stageC(b, TT):
            """scores -> E=exp, Z=rowsum, alpha=E/Z; alpha^T via PE transpose."""
            AT = aT_pool.tile([P, 4, L], b16, tag="aT")
            Z = zz_pool.tile([P, 4], f32, tag="zz")
            RZ = zz_pool.tile([P, 4], f32, tag="rz")
            pend = None  # deferred PE-transpose work to hide ACT/DVE latency
            def flush(pend):
                Ab, lc = pend
                ls = slice(lc * P, (lc + 1) * P)
                pat = p_sc.tile([P, 4, P], b16, tag="p_sc", name="pat")
                for wc in range(4):
                    nc.tensor.transpose(
                        pat[:, wc, :], Ab[:, wc * P : (wc + 1) * P], ident[:]
                    )
                nc.vector.tensor_copy(out=AT[:, :, ls], in_=pat[:])
            for lc in range(4):
                ls = slice(lc * P, (lc + 1) * P)
                ps = p_sc.tile([P, L], f32, tag="p_sc")
                for ec in range(8):
                    nc.tensor.matmul(
                        ps[:], TT[:, ec, ls], w_outT[:, ec, :],
                        start=(ec == 0), stop=(ec == 7),
                    )
                Eb = e_pool.tile([P, L], b16, tag="eb")
                nc.scalar.activation(Eb[:], ps[:], Exp, accum_out=Z[:, lc : lc + 1])
                nc.vector.reciprocal(RZ[:, lc : lc + 1], Z[:, lc : lc + 1])
                Ab = ab_pool.tile([P, L], b16, tag="ab")
                nc.vector.tensor_scalar_mul(Ab[:], Eb[:], RZ[:, lc : lc + 1])
                af = af_pool.tile([P, L], f32, tag="af")
                nc.vector.tensor_scalar_mul(af[:], Eb[:], RZ[:, lc : lc + 1])
                nc.sync.dma_start(out=alpha_ext[b, ls, :], in_=af[:])
                if pend is not None:
                    flush(pend)
                pend = (Ab, lc)
            flush(pend)
            return AT

        def stageD(b, CB, AT):
            """mixT[dd, l] = sum_w C[w, dd] * alphaT[w, l], dd on partitions."""
            MT = mT_pool.tile([P, 8, L], b16, tag="mT")
            for dc in range(8):
                ds_ = slice(dc * P, (dc + 1) * P)
                ps = p_mx.tile([P, L], f32, tag="p_mx")
                for wc in range(4):
                    nc.tensor.matmul(
                        ps[:], CB[:, wc, ds_], AT[:, wc, :],
                        start=(wc == 0), stop=(wc == 3),
                    )
                nc.vector.tensor_copy(out=MT[:, dc, :], in_=ps[:])
            return MT

        def stageE(b, CT, MT):
            """out[l, dd] = mix @ Wfc1^T + c @ Wfc2^T + b_fc."""
            for lc in range(4):
                ls = slice(lc * P, (lc + 1) * P)
                for dhalf in range(2):
                    dds = slice(dhalf * 512, (dhalf + 1) * 512)
                    ps = p_out.tile([P, 512], f32, tag="p_out")
                    for ec in range(8):
                        nc.tensor.matmul(
                            ps[:], MT[:, ec, ls], w_fcT[:, ec, dds],
                            start=(ec == 0), stop=False,
                        )
                    for ec in range(8):
                        nc.tensor.matmul(
                            ps[:], CT[:, ec, ls], w_fcT[:, 8 + ec, dds],
                            start=False, stop=(ec == 7),
                        )
                    of = of_pool.tile([P, 512], f32, tag="of")
                    nc.vector.tensor_tensor(of[:], ps[:], bfc[:, dds], ADD)
                    nc.sync.dma_start(out=out_ext[b, ls, dds], in_=of[:])

        # ---- emission: W_in/W_in2 + elem-0 inputs first so stage B can
        # start ASAP; remaining weights prepped while B(0) runs.
        prep_weight(w_inT, w_in_ext, 8, D)
        prep_weight(w_in2T, w_in2_ext, 8, D)
        A0 = stageA(0)

        prep_weight(w_outT, w_out_ext, 4, D)
        prep_weight(w_fcT, w_fc_ext, 8, 2 * D)

        # biases (tiny, copy-mode)
        bsum = wpool.tile([P, 8], f32)
        b1 = wpool.tile([P, 8], f32)
        b2 = wpool.tile([P, 8], f32)
        with nc.allow_non_contiguous_dma(reason="tiny one-time bias load"):
            nc.sync.dma_start(out=b1[:], in_=b_in_ext.rearrange("(c p) -> p c", p=P))
            nc.sync.dma_start(out=b2[:], in_=b_in2_ext.rearrange("(c p) -> p c", p=P))
        nc.vector.tensor_tensor(bsum[:], b1[:], b2[:], ADD)
        bfc = wpool.tile([P, D], f32)
        nc.sync.dma_start(out=bfc[0:1, :], in_=b_fc_ext[None, :])
        k = 1
        while k < P:
            nc.sync.dma_start(out=bfc[k : 2 * k, :], in_=bfc[0:k, :])
            k *= 2

        # software pipeline: defer D/E one element so the PE never waits on
        # the softmax chain — B/C of element b+1 fill the gap.
        prev = None
        for b in range(BPC):
            QT, CT, CB = A0 if b == 0 else stageA(b)
            TT = stageB(b, QT, CT)
            AT = stageC(b, TT)
            if prev is not None:
                pb, pCB, pAT, pCT = prev
                MT = stageD(pb, pCB, pAT)
                stageE(pb, pCT, MT)
            prev = (b, CB, AT, CT)
        pb, pCB, pAT, pCT = prev
        MT = stageD(pb, pCB, pAT)
        stageE(pb, pCT, MT)

    orig = nc.to_json_bytes
    nc.to_json_bytes = lambda: _split_waits_json(orig())
    return nc


def kernel(**inputs):
    from concourse.bass_utils import run_bass_kernel_spmd

    if "nc" not in _cached:
        _cached["nc"] = _build()
    nc = _cached["nc"]

    ins = {k: np.asarray(v, dtype=np.float32) for k, v in inputs.items()}
    in_maps = []
    for i in range(NCORES):
        sl = slice(i * BPC, (i + 1) * BPC)
        in_maps.append(
            {
                "query": ins["query"][sl],
                "context": ins["context"][sl],
                "W_in": ins["W_in"],
                "b_in": ins["b_in"],
                "W_in2": ins["W_in2"],
                "b_in2": ins["b_in2"],
                "W_out": ins["W_out"],
                "W_fc": ins["W_fc"],
                "b_fc": ins["b_fc"],
            }
        )
    res = run_bass_kernel_spmd(nc, in_maps, core_ids=list(range(NCORES)))
    out = np.concatenate([res.results[i]["out"] for i in range(NCORES)], axis=0)
    alpha = np.concatenate([res.results[i]["alpha"] for i in range(NCORES)], axis=0)
    return out, alpha


# revision 5
# speedup vs baseline: 1.4348x; 1.4348x over previous
"""AddAttention (Bahdanau) kernel for one TRN2 chip (8 NeuronCores).

Data-parallel over batch: B=32 -> 4 batch elements per core, weights
replicated. Per element (L=512, D=1024):
    q = query @ W_in^T ; c = context @ W_in2^T
    T = tanh(q + c + b_in + b_in2)
    scores = T @ W_out^T ; alpha = softmax(scores)
    mix = alpha @ context
    out = [mix, context] @ W_fc^T + b_fc
All matmuls run in bf16 (fp32 PSUM accumulation). Inputs stream in through
gpsimd casting DMAs (f32 DRAM -> bf16 SBUF). The PE wants the contraction
axis on partitions, so several tensors are transposed on chip:
  - W_in/W_in2 and element-0 query/context: TensorEngine identity-transpose
    (the PE is idle during startup, and the DMA xbar serializes on every
    transpose<->copy mode switch, which would serialize the whole prologue);
  - W_out/W_fc and steady-state query/context: DMA xbar, with explicit
    scheduler deps so each transpose batch runs as one block (one mode
    switch per batch instead of one per transfer);
  - alpha (per l-chunk, mid-pipeline): TensorEngine identity-transpose.
fp32 outputs.
"""

import numpy as np

B, L, D = 32, 512, 1024
P = 128
NCORES = 8
BPC = B // NCORES  # batch elements per core

_cached = {}


# --- walrus workaround: this compiler build rejects >1 sync wait on several
# instruction classes. Rewrite the serialized BIR: surplus waits become
# single-wait NoOps immediately before the instruction on the same engine
# (engine streams are sequential, so semantics are unchanged).
def _split_waits_json(bir_bytes):
    import json

    bir = json.loads(bir_bytes)
    n = 0
    for f in bir["functions"]:
        for blk in f["blocks"]:
            out = []
            for inst in blk["instructions"]:
                si = inst.get("sync_info")
                waits = (si or {}).get("on_wait") or []
                if len(waits) > 1:
                    for w in waits[:-1]:
                        n += 1
                        out.append(
                            {
                                "debug": inst.get("debug"),
                                "engine": inst["engine"],
                                "ins": [],
                                "name": f"I-wsplit-{n}",
                                "opcode": "NoOp",
                                "outs": [],
                                "sync_info": {"on_update": [], "on_wait": [w]},
                                "text_hint": "wait_split",
                            }
                        )
                    si["on_wait"] = [waits[-1]]
                out.append(inst)
            blk["instructions"] = out
    return json.dumps(bir).encode()


def _build():
    import concourse.bass as bass
    import concourse.tile as tile
    from concourse import mybir
    from concourse.masks import make_identity
    from concourse.tile import add_dep_helper
    from contextlib import ExitStack

    f32 = mybir.dt.float32
    b16 = mybir.dt.bfloat16
    Tanh = mybir.ActivationFunctionType.Tanh
    Exp = mybir.ActivationFunctionType.Exp
    ADD = mybir.AluOpType.add

    nc = bass.Bass("TRN2", target_bir_lowering=False, debug=False, num_devices=NCORES)

    q_ext = nc.dram_tensor("query", [BPC, L, D], f32, kind="ExternalInput").ap()
    c_ext = nc.dram_tensor("context", [BPC, L, D], f32, kind="ExternalInput").ap()
    w_in_ext = nc.dram_tensor("W_in", [D, D], f32, kind="ExternalInput").ap()
    b_in_ext = nc.dram_tensor("b_in", [D], f32, kind="ExternalInput").ap()
    w_in2_ext = nc.dram_tensor("W_in2", [D, D], f32, kind="ExternalInput").ap()
    b_in2_ext = nc.dram_tensor("b_in2", [D], f32, kind="ExternalInput").ap()
    w_out_ext = nc.dram_tensor("W_out", [L, D], f32, kind="ExternalInput").ap()
    w_fc_ext = nc.dram_tensor("W_fc", [D, 2 * D], f32, kind="ExternalInput").ap()
    b_fc_ext = nc.dram_tensor("b_fc", [D], f32, kind="ExternalInput").ap()
    out_ext = nc.dram_tensor("out", [BPC, L, D], f32, kind="ExternalOutput").ap()
    alpha_ext = nc.dram_tensor("alpha", [BPC, L, L], f32, kind="ExternalOutput").ap()

    with tile.TileContext(nc) as tc, ExitStack() as ctx:
        pool = lambda name, bufs, space="SBUF": ctx.enter_context(
            tc.tile_pool(name=name, bufs=bufs, space=space)
        )
        wpool = pool("weights", 1)
        wstg = pool("wstg", 1)      # bf16 natural weight staging (transpose src)
        qn_pool = pool("qn", 2)     # query natural bf16 (transpose src)
        cb_pool = pool("cb", 2)     # context natural bf16 (kept for stage D)
        qT_pool = pool("qT", 1)
        cT_pool = pool("cT", 2)     # read by deferred stage E
        tt_pool = pool("tt", 1)
        e_pool = pool("eb", 2)
        ab_pool = pool("ab", 2)
        aT_pool = pool("aT", 2)     # read by deferred stage D
        mT_pool = pool("mT", 1)
        zz_pool = pool("zz", 2)
        af_pool = pool("af", 2)
        of_pool = pool("of", 2)
        p_tt = pool("p_tt", 2, "PSUM")
        p_sc = pool("p_sc", 2, "PSUM")   # scores + PE alpha-transpose tiles
        p_mx = pool("p_mx", 2, "PSUM")   # mixT + PE prologue-transpose tiles
        p_out = pool("p_out", 2, "PSUM")

        ident = wpool.tile([P, P], b16)
        make_identity(nc, ident[:])

        # final transposed weights: [P (col % P), cols//P, rows]
        w_inT = wpool.tile([P, 8, D], b16)    # W_in^T  [d, e]
        w_in2T = wpool.tile([P, 8, D], b16)   # W_in2^T [d, e]
        w_outT = wpool.tile([P, 8, L], b16)   # W_out^T [e, w]
        w_fcT = wpool.tile([P, 16, D], b16)   # W_fc^T  [f, dd]

        def pe_transpose(dst3, src2, ncol_chunks, rslice):
            """dst3[:, cc, rslice] = src2[:, cc*P:(cc+1)*P].T via PE, grouped
            4 transposes per PSUM tile + one DVE copy per group."""
            for g in range(0, ncol_chunks, 4):
                gw = min(4, ncol_chunks - g)
                pat = p_mx.tile([P, 4, P], b16, tag="p_mx", name="pat_pre")
                for j in range(gw):
                    cc = g + j
                    nc.tensor.transpose(
                        pat[:, j, :], src2[:, cc * P : (cc + 1) * P], ident[:]
                    )
                nc.vector.tensor_copy(
                    out=dst3[:, g : g + gw, rslice], in_=pat[:, :gw, :]
                )

        def prep_weight_pe(dst, src, nrow_chunks, cols):
            """Casting loads, then PE transposes (for the prologue: PE idle)."""
            stage = wstg.tile([P, 8, 2 * D], b16, tag="wstg", name="wstg")
            for rc in range(nrow_chunks):
                nc.gpsimd.dma_start(
                    out=stage[:, rc, :cols], in_=src[rc * P : (rc + 1) * P, :]
                )
            for rc in range(nrow_chunks):
                pe_transpose(
                    dst, stage[:, rc, :cols], cols // P,
                    slice(rc * P, (rc + 1) * P),
                )

        def prep_weight_xbar(dst, src, nrow_chunks, cols):
            """Casting loads, then one batched block of xbar transposes."""
            stage = wstg.tile([P, 8, 2 * D], b16, tag="wstg", name="wstg")
            loads = []
            for rc in range(nrow_chunks):
                loads.append(
                    nc.gpsimd.dma_start(
                        out=stage[:, rc, :cols], in_=src[rc * P : (rc + 1) * P, :]
                    )
                )
            for rc in range(nrow_chunks):
                t = nc.sync.dma_start_transpose(
                    dst[:, :, rc * P : (rc + 1) * P], stage[:, rc, :cols]
                )
                for ld in loads:
                    add_dep_helper(t.ins, ld.ins, sync=False,
                                   reason="xbar batch after all loads")

        # ---- per-element stages ----
        def stageA(b, use_pe):
            """Casting loads then transposes -> QT/CT; CB kept natural."""
            QN = qn_pool.tile([P, 4, D], b16, tag="qn")
            CB = cb_pool.tile([P, 4, D], b16, tag="cb")
            QT = qT_pool.tile([P, 8, L], b16, tag="qT")
            CT = cT_pool.tile([P, 8, L], b16, tag="cT")
            loads = []
            for lc in range(4):
                ls = slice(lc * P, (lc + 1) * P)
                loads.append(nc.gpsimd.dma_start(out=QN[:, lc, :], in_=q_ext[b, ls, :]))
                loads.append(nc.gpsimd.dma_start(out=CB[:, lc, :], in_=c_ext[b, ls, :]))
            if use_pe:
                for lc in range(4):
                    ls = slice(lc * P, (lc + 1) * P)
                    pe_transpose(QT, QN[:, lc, :], 8, ls)
                    pe_transpose(CT, CB[:, lc, :], 8, ls)
            else:
                for lc in range(4):
                    ls = slice(lc * P, (lc + 1) * P)
                    for srcrow, dst in ((QN[:, lc, :], QT), (CB[:, lc, :], CT)):
                        t = nc.sync.dma_start_transpose(dst[:, :, ls], srcrow)
                        for ld in loads:
                            add_dep_helper(t.ins, ld.ins, sync=False,
                                           reason="xbar batch after all loads")
            return QT, CT, CB

        def stageB(b, QT, CT):
            """TT[e, l] = tanh(W_in q^T + W_in2 c^T + bias), e on partitions."""
            TT = tt_pool.tile([P, 8, L], b16, tag="tt")
            for ec in range(8):
                es = slice(ec * P, (ec + 1) * P)
                ps = p_tt.tile([P, L], f32, tag="p_tt")
                for dc in range(8):
                    nc.tensor.matmul(
                        ps[:], w_inT[:, dc, es], QT[:, dc, :],
                        start=(dc == 0), stop=False,
                    )
                for dc in range(8):
                    nc.tensor.matmul(
                        ps[:], w_in2T[:, dc, es], CT[:, dc, :],
                        start=False, stop=(dc == 7),
                    )
                nc.scalar.activation(
                    TT[:, ec, :], ps[:], Tanh, bias=bsum[:, ec : ec + 1]
                )
            return TT

        def stageC(b, TT):
            """scores -> E=exp, Z=rowsum, alpha=E/Z; alpha^T via PE transpose."""
            AT = aT_pool.tile([P, 4, L], b16, tag="aT")
            Z = zz_pool.tile([P, 4], f32, tag="zz")
            RZ = zz_pool.tile([P, 4], f32, tag="rz")
            pend = None  # deferred PE-transpose work to hide ACT/DVE latency
            def flush(pend):
                Ab, lc = pend
                ls = slice(lc * P, (lc + 1) * P)
                pat = p_sc.tile([P, 4, P], b16, tag="p_sc", name="pat")
                for wc in range(4):
                    nc.tensor.transpose(
                        pat[:, wc, :], Ab[:, wc * P : (wc + 1) * P], ident[:]
                    )
                nc.vector.tensor_copy(out=AT[:, :, ls], in_=pat[:])
            for lc in range(4):
                ls = slice(lc * P, (lc + 1) * P)
                ps = p_sc.tile([P, L], f32, tag="p_sc")
                for ec in range(8):
                    nc.tensor.matmul(
                        ps[:], TT[:, ec, ls], w_outT[:, ec, :],
                        start=(ec == 0), stop=(ec == 7),
                    )
                Eb = e_pool.tile([P, L], b16, tag="eb")
                nc.scalar.activation(Eb[:], ps[:], Exp, accum_out=Z[:, lc : lc + 1])
                nc.vector.reciprocal(RZ[:, lc : lc + 1], Z[:, lc : lc + 1])
                Ab = ab_pool.tile([P, L], b16, tag="ab")
                nc.vector.tensor_scalar_mul(Ab[:], Eb[:], RZ[:, lc : lc + 1])
                af = af_pool.tile([P, L], f32, tag="af")
                nc.vector.tensor_scalar_mul(af[:], Eb[:], RZ[:, lc : lc + 1])
                nc.sync.dma_start(out=alpha_ext[b, ls, :], in_=af[:])
                if pend is not None:
                    flush(pend)
                pend = (Ab, lc)
            flush(pend)
            return AT

        def stageD(b, CB, AT):
            """mixT[dd, l] = sum_w C[w, dd] * alphaT[w, l], dd on partitions."""
            MT = mT_pool.tile([P, 8, L], b16, tag="mT")
            for dc in range(8):
                ds_ = slice(dc * P, (dc + 1) * P)
                ps = p_mx.tile([P, L], f32, tag="p_mx")
                for wc in range(4):
                    nc.tensor.matmul(
                        ps[:], CB[:, wc, ds_], AT[:, wc, :],
                        start=(wc == 0), stop=(wc == 3),
                    )
                nc.vector.tensor_copy(out=MT[:, dc, :], in_=ps[:])
            return MT

        def stageE(b, CT, MT):
            """out[l, dd] = mix @ Wfc1^T + c @ Wfc2^T + b_fc."""
            for lc in range(4):
                ls = slice(lc * P, (lc + 1) * P)
                for dhalf in range(2):
                    dds = slice(dhalf * 512, (dhalf + 1) * 512)
                    ps = p_out.tile([P, 512], f32, tag="p_out")
                    for ec in range(8):
                        nc.tensor.matmul(
                            ps[:], MT[:, ec, ls], w_fcT[:, ec, dds],
                            start=(ec == 0), stop=False,
                        )
                    for ec in range(8):
                        nc.tensor.matmul(
                            ps[:], CT[:, ec, ls], w_fcT[:, 8 + ec, dds],
                            start=False, stop=(ec == 7),
                        )
                    of = of_pool.tile([P, 512], f32, tag="of")
                    nc.vector.tensor_tensor(of[:], ps[:], bfc[:, dds], ADD)
                    nc.sync.dma_start(out=out_ext[b, ls, dds], in_=of[:])

        # ---- prologue: everything stage B(0) needs goes through the PE
        # (idle anyway); later weights go through the xbar while B(0) runs.
        A0 = stageA(0, use_pe=True)
        prep_weight_pe(w_inT, w_in_ext, 8, D)
        prep_weight_pe(w_in2T, w_in2_ext, 8, D)
        prep_weight_xbar(w_outT, w_out_ext, 4, D)
        prep_weight_xbar(w_fcT, w_fc_ext, 8, 2 * D)

        # biases (tiny, copy-mode)
        bsum = wpool.tile([P, 8], f32)
        b1 = wpool.tile([P, 8], f32)
        b2 = wpool.tile([P, 8], f32)
        with nc.allow_non_contiguous_dma(reason="tiny one-time bias load"):
            nc.sync.dma_start(out=b1[:], in_=b_in_ext.rearrange("(c p) -> p c", p=P))
            nc.sync.dma_start(out=b2[:], in_=b_in2_ext.rearrange("(c p) -> p c", p=P))
        nc.vector.tensor_tensor(bsum[:], b1[:], b2[:], ADD)
        bfc = wpool.tile([P, D], f32)
        nc.sync.dma_start(out=bfc[0:1, :], in_=b_fc_ext[None, :])
        k = 1
        while k < P:
            nc.sync.dma_start(out=bfc[k : 2 * k, :], in_=bfc[0:k, :])
            k *= 2

        # software pipeline: defer D/E one element so the PE never waits on
        # the softmax chain — B/C of element b+1 fill the gap.
        prev = None
        for b in range(BPC):
            QT, CT, CB = A0 if b == 0 else stageA(b, use_pe=False)
            TT = stageB(b, QT, CT)
            AT = stageC(b, TT)
            if prev is not None:
                pb, pCB, pAT, pCT = prev
                MT = stageD(pb, pCB, pAT)
                stageE(pb, pCT, MT)
            prev = (b, CB, AT, CT)
        pb, pCB, pAT, pCT = prev
        MT = stageD(pb, pCB, pAT)
        stageE(pb, pCT, MT)

    orig = nc.to_json_bytes
    nc.to_json_bytes = lambda: _split_waits_json(orig())
    return nc


def kernel(**inputs):
    from concourse.bass_utils import run_bass_kernel_spmd

    if "nc" not in _cached:
        _cached["nc"] = _build()
    nc = _cached["nc"]

    ins = {k: np.asarray(v, dtype=np.float32) for k, v in inputs.items()}
    in_maps = []
    for i in range(NCORES):
        sl = slice(i * BPC, (i + 1) * BPC)
        in_maps.append(
            {
                "query": ins["query"][sl],
                "context": ins["context"][sl],
                "W_in": ins["W_in"],
                "b_in": ins["b_in"],
                "W_in2": ins["W_in2"],
                "b_in2": ins["b_in2"],
                "W_out": ins["W_out"],
                "W_fc": ins["W_fc"],
                "b_fc": ins["b_fc"],
            }
        )
    res = run_bass_kernel_spmd(nc, in_maps, core_ids=list(range(NCORES)))
    out = np.concatenate([res.results[i]["out"] for i in range(NCORES)], axis=0)
    alpha = np.concatenate([res.results[i]["alpha"] for i in range(NCORES)], axis=0)
    return out, alpha
